# revision 1
# baseline (speedup 1.0000x reference)
"""MoE top-2 routing kernel for Trainium2, expert-parallel across 8 NeuronCores.

Problem (hardcoded): x [4, 2048, 1024] f32, gate_w [1024, 8], w1 [8, 1024, 4096],
w2 [8, 4096, 1024], TOP_K=2, exact GELU, softmax-renormalized top-2 combine.

Strategy: each core owns one expert. x is replicated; every core computes the
router for all 8192 tokens (cheap), compacts the token ids routed to its own
expert with gpsimd sparse_gather, gathers those rows of x by indirect DMA,
runs the expert MLP in transposed space (hT = gelu(w1.T @ x.T), yT = w2.T @ hT),
transposes back, scales by the combine weight and scatters into a per-core
output. Host sums the 8 per-core outputs (each token appears on exactly the
two cores of its top-2 experts).
"""

import numpy as np
from contextlib import ExitStack

import concourse.bass as bass
import concourse.mybir as mybir
import concourse.tile as tile
from concourse import bacc, library_config
from concourse.bass_utils import run_bass_kernel_spmd

P = 128
B, T, C, E, F = 4, 2048, 1024, 8, 4096
NT = B * T              # 8192 tokens
NTILE = NT // P         # 64 token tiles
CB = C // P             # 8 contraction blocks over C
FB = F // P             # 32 blocks over F
CAP = 2560              # per-expert token capacity (max n_e); 5 passes of 512
PASS_N = 512
NPASS = CAP // PASS_N   # 5
CAPT = CAP // P         # 20 gather tiles
CAPW = CAP // 16        # 160 wrapped cols
DUMP = NT               # dump row index (x_pad[NT] == 0)

f32 = mybir.dt.float32
f32r = mybir.dt.float32r
i32 = mybir.dt.int32
u32 = mybir.dt.uint32

bf16 = mybir.dt.bfloat16
MM_DTYPE = "f32"  # "f32" (exact, 4 cyc/row) or "bf16" (1 cyc/row, ~3e-3 rel err)


def _src_salt():
    """Shape-salt derived from this file's source so every kernel edit changes
    the executable signature through every cache layer (client NEFF cache and
    any terminal-side executable cache)."""
    import hashlib

    with open(__file__, "rb") as f:
        h = int(hashlib.sha256(f.read()).hexdigest(), 16)
    return 1 + (h % 509)


def build_nc(repeats=1, mm=None):
    """repeats: trace the whole program body N times back-to-back (straight-
    line); used to measure per-invocation HW time by differencing."""
    mm = mm or MM_DTYPE
    mdt = f32 if mm == "f32" else bf16
    nc = bacc.Bacc("TRN2", target_bir_lowering=False, debug=False, num_devices=8)

    salt_in = nc.declare_dram_parameter(
        "salt",
        [1, _src_salt() + 509 * (repeats - 1) + 2039 * (mm != "f32")],
        f32,
        isOutput=False,
    )
    x_in = nc.declare_dram_parameter("x", [NT + 1, C], f32, isOutput=False)
    gw_in = nc.declare_dram_parameter("gw", [P, CB * E], f32, isOutput=False)
    w1_in = nc.declare_dram_parameter("w1t", [FB, P, CB, P], mdt, isOutput=False)
    w2_in = nc.declare_dram_parameter("w2t", [CB, P, FB, P], mdt, isOutput=False)
    ident_in = nc.declare_dram_parameter("ident", [P, P], f32, isOutput=False)
    tokid1_in = nc.declare_dram_parameter("tokid1", [P, NTILE], f32, isOutput=False)
    eown_in = nc.declare_dram_parameter("eown", [P, 1], f32, isOutput=False)
    yout = nc.declare_dram_parameter("yout", [NT + 1, C], f32, isOutput=True)

    with tile.TileContext(nc) as tc, ExitStack() as ctx:
        pers = ctx.enter_context(tc.tile_pool(name="pers", bufs=1))
        rt = ctx.enter_context(tc.tile_pool(name="rt", bufs=3))
        xap = ctx.enter_context(tc.tile_pool(name="xa", bufs=3))
        xtp = ctx.enter_context(tc.tile_pool(name="xt", bufs=2))
        xgtp = ctx.enter_context(tc.tile_pool(name="xgt", bufs=1))
        htp = ctx.enter_context(tc.tile_pool(name="ht", bufs=1))
        w1p = ctx.enter_context(tc.tile_pool(name="w1p", bufs=3))
        w2p = ctx.enter_context(tc.tile_pool(name="w2p", bufs=2))
        ytp = ctx.enter_context(tc.tile_pool(name="ytp", bufs=2))
        yap = ctx.enter_context(tc.tile_pool(name="yap", bufs=5))
        psp = ctx.enter_context(tc.tile_pool(name="ps", bufs=2, space="PSUM"))

        for _rep in range(repeats):
            lib_inst = nc.gpsimd.load_library(library_config.sparse_gather)

            ident = pers.tile([P, P], f32)
            nc.sync.dma_start(ident[:], ident_in[:])
            gw_sb = pers.tile([P, CB * E], f32)
            nc.sync.dma_start(gw_sb[:], gw_in[:])
            tokid1 = pers.tile([P, NTILE], f32)
            nc.sync.dma_start(tokid1[:], tokid1_in[:])
            eown = pers.tile([P, 1], f32)
            nc.sync.dma_start(eown[:], eown_in[:])

            # ---------------- router ----------------
            M8 = pers.tile([P, NTILE, 8], f32)
            I8 = pers.tile([P, NTILE, 8], u32)
            for j in range(NTILE):
                xa = xap.tile([P, C], f32, tag="xa")
                nc.sync.dma_start(xa[:], x_in[j * P : (j + 1) * P, :])
                xT = xtp.tile([P, CB, P], f32, tag="xT")
                for b in range(CB):
                    tp = psp.tile([P, P], f32, space="PSUM", tag="T")
                    nc.tensor.transpose(tp[:], xa[:, b * P : (b + 1) * P], ident[:])
                    nc.vector.tensor_copy(xT[:, b, :], tp[:])
                L = psp.tile([P, 8], f32, space="PSUM", tag="Y")
                for b in range(CB):
                    nc.tensor.matmul(
                        L[:],
                        lhsT=xT[:, b, :],
                        rhs=gw_sb[:, b * E : b * E + E],
                        start=(b == 0),
                        stop=(b == CB - 1),
                    )
                Lc = rt.tile([P, 8], f32, tag="Lc")
                nc.vector.tensor_copy(Lc[:], L[:])
                nc.vector.max_with_indices(M8[:, j, :], I8[:, j, :], Lc[:])

            # batched router math -> per-token selection + combine weight for e_own
            CW1 = pers.tile([P, NTILE], f32)
            SEL = pers.tile([P, NTILE], f32)
            val1 = pers.tile([P, NTILE], f32)
            val2 = pers.tile([P, NTILE], f32)
            d = rt.tile([P, NTILE], f32, tag="rm")
            nc.vector.tensor_tensor(d[:], M8[:, :, 0], M8[:, :, 1], op=mybir.AluOpType.subtract)
            nc.scalar.activation(CW1[:], d[:], mybir.ActivationFunctionType.Sigmoid)
            if1 = rt.tile([P, NTILE], f32, tag="rm")
            if2 = rt.tile([P, NTILE], f32, tag="rm")
            nc.vector.tensor_copy(if1[:], I8[:, :, 0])
            nc.vector.tensor_copy(if2[:], I8[:, :, 1])
            se1 = rt.tile([P, NTILE], f32, tag="rm")
            se2 = rt.tile([P, NTILE], f32, tag="rm")
            # se1 = (if1 == e_own), se2 = (if2 == e_own)   (per-partition scalar AP)
            nc.vector.tensor_scalar(se1[:], if1[:], eown[:, 0:1], None, op0=mybir.AluOpType.is_equal)
            nc.vector.tensor_scalar(se2[:], if2[:], eown[:, 0:1], None, op0=mybir.AluOpType.is_equal)
            nc.vector.tensor_add(SEL[:], se1[:], se2[:])
            # CWE = se1*cw1 + se2*(1-cw1)
            t1 = rt.tile([P, NTILE], f32, tag="rm")
            t2 = rt.tile([P, NTILE], f32, tag="rm")
            CWE = rt.tile([P, NTILE], f32, tag="rm")
            nc.vector.tensor_mul(t1[:], se1[:], CW1[:])
            nc.vector.tensor_mul(t2[:], se2[:], CW1[:])
            nc.vector.tensor_sub(t2[:], se2[:], t2[:])
            nc.vector.tensor_add(CWE[:], t1[:], t2[:])
            # val1 = SEL * (tokid+1) - 1 ; val2 = SEL * (CWE+1) - 1
            nc.vector.tensor_mul(val1[:], SEL[:], tokid1[:])
            nc.vector.tensor_scalar(val1[:], val1[:], 1.0, None, op0=mybir.AluOpType.subtract)
            nc.vector.tensor_scalar(t1[:], CWE[:], 1.0, None, op0=mybir.AluOpType.add)
            nc.vector.tensor_mul(val2[:], SEL[:], t1[:])
            nc.vector.tensor_scalar(val2[:], val2[:], 1.0, None, op0=mybir.AluOpType.subtract)

            # ---------------- compaction ----------------
            v1w = pers.tile([16, NT // 16], f32)
            v2w = pers.tile([16, NT // 16], f32)
            for q in range(8):
                nc.sync.dma_start(v1w[0:16, q * NTILE : (q + 1) * NTILE], val1[16 * q : 16 * (q + 1), :])
                nc.sync.dma_start(v2w[0:16, q * NTILE : (q + 1) * NTILE], val2[16 * q : 16 * (q + 1), :])
            idsC = pers.tile([16, CAPW], f32)
            cwC = pers.tile([16, CAPW], f32)
            nf1 = pers.tile([1, 1], u32)
            nf2 = pers.tile([1, 1], u32)
            # HW sparse_gather writes only the found prefix; pre-fill the tail
            # marker ourselves (the simulator fills -1, hardware does not).
            nc.vector.memset(idsC[:], -1.0)
            nc.vector.memset(cwC[:], -1.0)
            sg1 = nc.gpsimd.sparse_gather(idsC[:], v1w[:], num_found=nf1[:])
            sg2 = nc.gpsimd.sparse_gather(cwC[:], v2w[:], num_found=nf2[:])
            from concourse.tile_rust import add_dep_helper

            add_dep_helper(sg1.ins, lib_inst.ins, reason="sparse_gather needs library")
            add_dep_helper(sg2.ins, lib_inst.ins, reason="sparse_gather needs library")
            # tail fill is -1: ids -> DUMP, cw -> 0
            neg = rt.tile([16, CAPW], f32, tag="fix")
            nc.vector.tensor_scalar(neg[:], idsC[:], 0.0, None, op0=mybir.AluOpType.is_lt)
            nc.vector.scalar_tensor_tensor(
                idsC[:], neg[:], float(DUMP + 1), idsC[:],
                op0=mybir.AluOpType.mult, op1=mybir.AluOpType.add,
            )
            nc.vector.tensor_scalar_max(cwC[:], cwC[:], 0.0)

            IDS = pers.tile([P, CAPT], f32)
            CWS = pers.tile([P, CAPT], f32)
            for r in range(8):
                nc.sync.dma_start(IDS[16 * r : 16 * (r + 1), :], idsC[0:16, r::8])
                nc.sync.dma_start(CWS[16 * r : 16 * (r + 1), :], cwC[0:16, r::8])
            IDSi = pers.tile([P, CAPT], i32)
            nc.vector.tensor_copy(IDSi[:], IDS[:])

            # ---------------- expert MLP over capacity passes ----------------
            for p in range(NPASS):
                xgT = xgtp.tile([P, CB, PASS_N], mdt, tag="xgT")
                for g in range(PASS_N // P):
                    k = p * (PASS_N // P) + g
                    xg = xap.tile([P, C], f32, tag="xa")
                    nc.gpsimd.indirect_dma_start(
                        out=xg[:],
                        out_offset=None,
                        in_=x_in[:],
                        in_offset=bass.IndirectOffsetOnAxis(ap=IDSi[:, k : k + 1], axis=0),
                    )
                    for b in range(CB):
                        tp = psp.tile([P, P], f32, space="PSUM", tag="T")
                        nc.tensor.transpose(tp[:], xg[:, b * P : (b + 1) * P], ident[:])
                        nc.vector.tensor_copy(xgT[:, b, g * P : (g + 1) * P], tp[:])

                hT = htp.tile([P, FB, PASS_N], mdt, tag="hT")
                for fb in range(FB):
                    w1sb = w1p.tile([P, CB * P], mdt, tag="w1")
                    nc.sync.dma_start(w1sb[:], w1_in[fb].rearrange("c b f -> c (b f)"))
                    psA = psp.tile([P, PASS_N], f32, space="PSUM", tag="A")
                    for b in range(CB):
                        nc.tensor.matmul(
                            psA[:],
                            lhsT=w1sb[:, b * P : (b + 1) * P],
                            rhs=xgT[:, b, :],
                            start=(b == 0),
                            stop=(b == CB - 1),
                        )
                    nc.scalar.activation(hT[:, fb, :], psA[:], mybir.ActivationFunctionType.Gelu)

                yasm = [
                    yap.tile([P, C], f32, tag="yasm", name=f"yasm_{_rep}_{p}_{g}")
                    for g in range(PASS_N // P)
                ]
                for cb in range(CB):
                    w2sb = w2p.tile([P, FB * P], mdt, tag="w2")
                    nc.sync.dma_start(w2sb[:], w2_in[cb].rearrange("f b c -> f (b c)"))
                    psY = psp.tile([P, PASS_N], f32, space="PSUM", tag="Y")
                    for fb in range(FB):
                        nc.tensor.matmul(
                            psY[:],
                            lhsT=w2sb[:, fb * P : (fb + 1) * P],
                            rhs=hT[:, fb, :],
                            start=(fb == 0),
                            stop=(fb == FB - 1),
                        )
                    yT = ytp.tile([P, PASS_N], f32, tag="yT")
                    nc.vector.tensor_copy(yT[:], psY[:])
                    for g in range(PASS_N // P):
                        k = p * (PASS_N // P) + g
                        tp = psp.tile([P, P], f32, space="PSUM", tag="T")
                        nc.tensor.transpose(tp[:], yT[:, g * P : (g + 1) * P], ident[:])
                        nc.vector.tensor_scalar_mul(
                            yasm[g][:, cb * P : (cb + 1) * P], tp[:], CWS[:, k : k + 1]
                        )
                for g in range(PASS_N // P):
                    k = p * (PASS_N // P) + g
                    # scatter-ADD: dump slots carry exact +/-0 rows, so wherever
                    # the hardware lands the out-of-range dump index (row 0 via the
                    # 25-bit DGE offset wrap; row NT in the simulator), adding
                    # zeros is harmless. Real token rows are unique per core and
                    # the output buffer starts zeroed, so add == write for them.
                    nc.gpsimd.indirect_dma_start(
                        out=yout[:],
                        out_offset=bass.IndirectOffsetOnAxis(ap=IDSi[:, k : k + 1], axis=0),
                        in_=yasm[g][:],
                        in_offset=None,
                        compute_op=mybir.AluOpType.add,
                    )

    nc.compile()
    return nc


def prep_inputs(x, gate_w, w1, w2, mm=None):
    """Host-side input prep. Returns per-core input maps."""
    import ml_dtypes

    mm = mm or MM_DTYPE
    wdt = np.float32 if mm == "f32" else ml_dtypes.bfloat16
    x = np.ascontiguousarray(np.asarray(x, dtype=np.float32)).reshape(NT, C)
    gate_w = np.asarray(gate_w, dtype=np.float32)
    w1 = np.asarray(w1, dtype=np.float32)
    w2 = np.asarray(w2, dtype=np.float32)

    x_pad = np.zeros((NT + 1, C), dtype=np.float32)
    x_pad[:NT] = x
    gw = np.ascontiguousarray(
        gate_w.reshape(CB, P, E).transpose(1, 0, 2).reshape(P, CB * E)
    )
    ident = np.eye(P, dtype=np.float32)
    tokid1 = (np.arange(NT, dtype=np.float32).reshape(NTILE, P).T + 1.0).copy()

    in_maps = []
    for e in range(E):
        w1t = np.ascontiguousarray(
            w1[e].reshape(CB, P, FB, P).transpose(2, 1, 0, 3).astype(wdt)
        )
        w2t = np.ascontiguousarray(
            w2[e].reshape(FB, P, CB, P).transpose(2, 1, 0, 3).astype(wdt)
        )
        in_maps.append(
            {
                "salt": np.zeros((1, _src_salt()), dtype=np.float32),
                "x": x_pad,
                "gw": gw,
                "w1t": w1t,
                "w2t": w2t,
                "ident": ident,
                "tokid1": tokid1,
                "eown": np.full((P, 1), float(e), dtype=np.float32),
            }
        )
    return in_maps


def check_capacity(x, gate_w):
    logits = x.reshape(NT, C).astype(np.float32) @ np.asarray(gate_w, np.float32)
    top2 = np.argpartition(-logits, 2, axis=1)[:, :2]
    counts = np.bincount(top2.ravel(), minlength=E)
    if counts.max() > CAP:
        raise RuntimeError(f"expert token count {counts.max()} exceeds CAP={CAP}")
    return counts


_NC = None


def _run_with_retries(nc, in_maps, attempts=4):
    """The first execution of a freshly-compiled NEFF occasionally fails with a
    transient runtime error (executable-load race in the remote terminal);
    subsequent executions succeed. Retry with backoff."""
    import time as _time

    last = None
    for i in range(attempts):
        try:
            return run_bass_kernel_spmd(nc, in_maps, list(range(E)))
        except Exception as e:  # jax.errors.JaxRuntimeError and friends
            last = e
            _time.sleep(5 + 15 * i)
    raise last


def kernel(x, gate_w, w1, w2):
    global _NC
    x = np.asarray(x)
    check_capacity(np.asarray(x, np.float32), gate_w)
    if _NC is None:
        _NC = build_nc()
    in_maps = prep_inputs(x, gate_w, w1, w2)
    res = _run_with_retries(_NC, in_maps)
    out = np.zeros((NT, C), dtype=np.float32)
    for e in range(E):
        out += res.results[e]["yout"][:NT]
    return out.reshape(B, T, C)

